# revision 35
# baseline (speedup 1.0000x reference)
"""GPT-NeoX attention (B=4, S=1024, D=2048, H=16) on 8 TRN2 NeuronCores.

Tensor-parallel over heads: 2 heads per core. Each core computes its slice
of the fused QKV projection, RoPE, causal attention, and writes the
transposed per-head output [hd, S]; the host concatenates heads.

Final design (344us baseline -> ~246us):
  - bf16 matmul operands everywhere (x, w, q/k, v, e); fp32 PSUM. Validated
    numerically: rel err ~5e-3 vs the 2e-2 gate. (fp8 q/k projection was
    measured at 5e-2 in simulation -- fails the gate; not used.)
  - Startup streaming: m-major weight DMAs + 4-chunk x DMAs so the first
    projection chain starts right after engine boot; a warm-up matmul chain
    keeps the PE clock ramp going until real data lands.
  - Attention software pipeline: scores matmuls issue LOOKAHEAD blocks
    ahead of the PV matmuls so the exp (ACT) latency never stalls PE.
  - Causal column restriction: diagonal blocks only compute score/exp/PV
    columns [off:512); fully-masked columns are zeroed via cheap Pool
    memsets of e; the triangular window is masked with one shared [128,128]
    lower-triangle multiply on DVE.
  - RoPE rotate-half via SBUF->SBUF DMA partition swap (sign folded into
    the sin table) instead of PE permutation matmuls.
  - Softmax denominators accumulate OFF the PE: per-block partial sums on
    Pool (cols 0:256) and DVE (cols 256:512) into one fp32r tile; a single
    ones-matmul per q-slice reduces over k partitions. The tail (ones-mm,
    reciprocal, normalize, out-DMA) is deferred one chain to hide latency.
  - proj(b+1) chains are interleaved between attn(b) chains so the PE
    stream stays dense while ACT catches up on exps.
"""

import os

import numpy as np
import ml_dtypes

import concourse.bass as bass
import concourse.tile as tile
from concourse import bacc, mybir

# Problem constants (contract: nn_GPTNeoXAttention, fixed shapes)
B, S, D = 4, 1024, 2048
H = 16
HD = 128  # head dim
NCORES = 8
HPC = H // NCORES  # heads per core
ROPE_BASE = 10000.0
T = B * S  # 4096 tokens
KC = D // 128  # 16 contraction chunks of the model dim
NSL = 512  # token-slice width per projection pass
NHALF = S // NSL  # 2 slices per batch
QCH = S // 512  # q slices per sequence in attention
SCALE = 1.0 / float(np.sqrt(HD))
LOOKAHEAD = 4  # scores blocks in flight ahead of PV consumption
NWARM = 40  # warm-up matmuls bridging the startup DMA window

F32 = mybir.dt.float32
F32R = mybir.dt.float32r
BF16 = mybir.dt.bfloat16
NP_BF16 = ml_dtypes.bfloat16

_CACHE = {}


def _build_program():
    nc = bacc.Bacc(
        "TRN2", target_bir_lowering=False, debug=False, num_devices=NCORES
    )

    # x8[p, bh, kc, s] = x[kc*128+p, bh*512+s]  (feature-major tokens)
    x_d = nc.dram_tensor("x8", [128, B * NHALF, KC, NSL], BF16,
                         kind="ExternalInput")
    # wqk[p, m, kc, f]: m in (q_h0, q_h1, k_h0, k_h1); lhsT chunks
    wqk_d = nc.dram_tensor("wqk", [128, 4, KC, 128], BF16,
                           kind="ExternalInput")
    # wv[p, kc, f]: rhs for the natural-layout v projection (2 heads x 128)
    wv_d = nc.dram_tensor("wv", [128, KC, 2 * HD], BF16,
                          kind="ExternalInput")
    bqk_d = nc.dram_tensor("bqk", [128, 4], F32, kind="ExternalInput")
    bv_d = nc.dram_tensor("bv", [128, 2 * HD], F32, kind="ExternalInput")
    cos_d = nc.dram_tensor("cosT", [128, S], BF16, kind="ExternalInput")
    # sinF is sign-folded: row i holds -sin for i<64, +sin for i>=64
    sin_d = nc.dram_tensor("sinF", [128, S], BF16, kind="ExternalInput")
    # tri[p, j] = 1 if j >= p else 0 (keep) -- shared diagonal-window mask
    tri_d = nc.dram_tensor("tri", [128, 128], BF16, kind="ExternalInput")
    ones_d = nc.dram_tensor("ones", [128, 128], F32R, kind="ExternalInput")
    ones16_d = nc.dram_tensor("ones16", [128, 128], BF16,
                              kind="ExternalInput")
    out_d = nc.dram_tensor("out", [HPC, HD, B, S], F32, kind="ExternalOutput")

    x_ap = x_d.ap()
    out_ap = out_d.ap()

    Exp = mybir.ActivationFunctionType.Exp
    Identity = mybir.ActivationFunctionType.Identity

    with tile.TileContext(nc) as tc:
        with (
            tc.tile_pool(name="singles", bufs=1) as singles,
            tc.tile_pool(name="xin", bufs=4) as xin_pool,
            tc.tile_pool(name="qk", bufs=8) as qk_pool,
            tc.tile_pool(name="vp", bufs=2) as v_pool,
            tc.tile_pool(name="qb", bufs=3) as qb_pool,
            tc.tile_pool(name="swp", bufs=3) as swp_pool,
            tc.tile_pool(name="expp", bufs=10) as exp_pool,
            tc.tile_pool(name="part", bufs=3) as part_pool,
            tc.tile_pool(name="outp", bufs=3) as out_pool,
            tc.tile_pool(name="rcp", bufs=2) as rcp_pool,
            tc.tile_pool(name="warmp", bufs=1) as warm_pool,
            tc.tile_pool(name="pp", bufs=6, space="PSUM") as pp,
            tc.tile_pool(name="po", bufs=2, space="PSUM") as po,
        ):
            # ---- input DMAs (priority order: first x half, then weights) --
            wqk_sb = singles.tile([128, 4, KC, 128], BF16)
            wv_sb = singles.tile([128, KC, 2 * HD], BF16)
            bqk_sb = singles.tile([128, 4], F32)
            bv_sb = singles.tile([128, 2 * HD], F32)
            cos_sb = singles.tile([128, S], BF16)
            sin_sb = singles.tile([128, S], BF16)
            tri_sb = singles.tile([128, 128], BF16)
            ones_sb = singles.tile([128, 128], F32R)
            ones16_sb = singles.tile([128, 128], BF16)

            # ---- PE warm-up: hold the clock ramp while DMA streams in ----
            warm_sb = warm_pool.tile([128, 512], BF16)
            nc.gpsimd.memset(warm_sb, 0.0)
            ps_warm = pp.tile([128, 512], F32, tag="ps")
            for _ in range(NWARM):
                nc.tensor.matmul(
                    ps_warm, warm_sb[:, :128], warm_sb, start=True, stop=True
                )

            # x(b0,h0) chunks first on the sync hwdge queue; the big weight
            # DMAs on the scalar hwdge queue; small constants on gpsimd
            # (software DGE). Keeps the startup-critical transfers unblocked.
            xsb0 = xin_pool.tile([128, KC, NSL], BF16, tag="x")
            for c in range(4):
                nc.sync.dma_start(
                    out=xsb0[:, 4 * c : 4 * c + 4, :],
                    in_=x_ap[:, 0, 4 * c : 4 * c + 4, :],
                )
            nc.scalar.dma_start(out=wqk_sb[:, 0], in_=wqk_d.ap()[:, 0])
            nc.gpsimd.dma_start(out=cos_sb, in_=cos_d.ap())
            nc.gpsimd.dma_start(out=sin_sb, in_=sin_d.ap())
            nc.gpsimd.dma_start(out=tri_sb, in_=tri_d.ap())
            nc.gpsimd.dma_start(out=ones_sb, in_=ones_d.ap())
            nc.gpsimd.dma_start(out=ones16_sb, in_=ones16_d.ap())
            nc.gpsimd.dma_start(out=bqk_sb, in_=bqk_d.ap())
            nc.gpsimd.dma_start(out=bv_sb, in_=bv_d.ap())
            # later weight chunks via the slower software-DGE dispatch path:
            # their natural dispatch delay keeps the first x half + wqk m0
            # at full HBM bandwidth during the startup-critical window
            for m in range(1, 4):
                nc.gpsimd.dma_start(out=wqk_sb[:, m], in_=wqk_d.ap()[:, m])
            nc.gpsimd.dma_start(out=wv_sb, in_=wv_d.ap())
            # drain the warm-up psum (GPSIMD cannot access PSUM; DVE is idle
            # at startup, and its first real op comes well after this)
            warm_out = warm_pool.tile([128, 1], F32)
            nc.vector.tensor_copy(warm_out, ps_warm[:, 0:1])

            # per-batch state created by the projection pieces
            qk_tiles = [None] * B  # [b] -> list of 4 [128, S] rotated q/k
            v_tiles = [None] * B  # [b] -> [128, 8, 256] natural v
            x_tiles = {}  # (b, half) -> xin tile

            def emit_x_dma(b, half):
                bh = b * NHALF + half
                if bh == 0:
                    xsb = xsb0
                else:
                    xsb = xin_pool.tile([128, KC, NSL], BF16, tag="x")
                    for c in range(4):
                        nc.sync.dma_start(
                            out=xsb[:, 4 * c : 4 * c + 4, :],
                            in_=x_ap[:, bh, 4 * c : 4 * c + 4, :],
                        )
                x_tiles[(b, half)] = xsb

            def proj_stream(b):
                """Generator emitting proj(b), yielding after each PE matmul
                so attention scheduling can interleave at instruction
                granularity."""
                qk_tiles[b] = [
                    qk_pool.tile([128, S], BF16, tag="qkt",
                                 name=f"qkt_{b}_{i}")
                    for i in range(4)
                ]
                v_tiles[b] = v_pool.tile(
                    [128, S // 128, 2 * HD], BF16, tag="v", name=f"v_{b}"
                )
                # both halves' x DMAs dispatch up front: on the sync queue
                # they must precede the rotate-swap dispatches, which block
                # waiting on ACT-produced qb tiles (head-of-line blocking
                # would otherwise delay the x transfers by a whole phase)
                emit_x_dma(b, 0)
                emit_x_dma(b, 1)
                for half in range(NHALF):
                    xsb = x_tiles[(b, half)]
                    sl = slice(half * NSL, (half + 1) * NSL)
                    for m in range(4):
                        ps = pp.tile([128, NSL], F32, tag="ps")
                        for kc in range(KC):
                            nc.tensor.matmul(
                                ps,
                                wqk_sb[:, m, kc, :],
                                xsb[:, kc, :],
                                start=(kc == 0),
                                stop=(kc == KC - 1),
                            )
                            yield
                        qb = qb_pool.tile([128, NSL], BF16, tag="qb")
                        nc.scalar.activation(
                            qb, ps, Identity, bias=bqk_sb[:, m : m + 1],
                            scale=1.0,
                        )
                        swp = swp_pool.tile([128, NSL], BF16, tag="sw")
                        nc.sync.dma_start(out=swp[0:64, :], in_=qb[64:128, :])
                        nc.sync.dma_start(out=swp[64:128, :], in_=qb[0:64, :])
                        dst = qk_tiles[b][m][:, sl]
                        nc.vector.tensor_mul(dst, qb, cos_sb[:, sl])
                        nc.vector.tensor_mul(swp, swp, sin_sb[:, sl])
                        nc.vector.tensor_add(dst, dst, swp)
                    for t in range(NSL // 128):
                        psv = pp.tile([128, NSL], F32, tag="ps")
                        for kc in range(KC):
                            nc.tensor.matmul(
                                psv[:, 0 : 2 * HD],
                                xsb[:, kc, t * 128 : (t + 1) * 128],
                                wv_sb[:, kc, :],
                                start=(kc == 0),
                                stop=(kc == KC - 1),
                            )
                            yield
                        nc.vector.tensor_add(
                            v_tiles[b][:, half * (NSL // 128) + t, :],
                            psv[:, 0 : 2 * HD],
                            bv_sb,
                        )

            def emit_attn_chain(b, h, qs, pull, pe_sum=False):
                """Emit one attention chain; returns the deferred tail.
                pull(n) interleaves up to n projection matmuls into the PE
                stream to keep it dense while ACT produces the exps.
                pe_sum accumulates the softmax denominator on the PE
                (for the final chain, where no cover for the off-engine
                partial chains exists)."""
                qT = qk_tiles[b][h]
                kT = qk_tiles[b][2 + h]
                nk = (qs * 512 + 512) // 128  # causal k chunks
                ps_out = po.tile([128, 512], F32)
                partial = None
                ps_sm_acc = None
                if pe_sum:
                    ps_sm_acc = pp.tile([128, 512], F32, tag="ps")
                else:
                    partial = part_pool.tile([128, 512], F32R, tag="pa")
                qsl0 = qs * 512
                e_tiles = [None] * nk

                def emit_scores(ki):
                    off = ki * 128 - qs * 512  # >=0 on diagonal
                    lo = max(off, 0)
                    pss = pp.tile([128, 512], F32, tag="ps")
                    nc.tensor.matmul(
                        pss[:, lo:512],
                        kT[:, ki * 128 : (ki + 1) * 128],
                        qT[:, qsl0 + lo : qsl0 + 512],
                        start=True,
                        stop=True,
                    )
                    # no zeroing of e[:, 0:lo): every consumer (PV matmul,
                    # partial sums, tri mask) is restricted to [lo:512)
                    e = exp_pool.tile([128, 512], BF16, tag="e")
                    nc.scalar.activation(
                        e[:, lo:512], pss[:, lo:512], Exp, scale=SCALE
                    )
                    if off >= 0:
                        nc.vector.tensor_mul(
                            e[:, off : off + 128],
                            e[:, off : off + 128],
                            tri_sb,
                        )
                    e_tiles[ki] = e

                def emit_pv(ki):
                    off = ki * 128 - qs * 512
                    lo = max(off, 0)
                    e = e_tiles[ki]
                    nc.tensor.matmul(
                        ps_out[:, lo:512],
                        v_tiles[b][:, ki, h * HD : (h + 1) * HD],
                        e[:, lo:512],
                        start=(ki == 0),
                        stop=(ki == nk - 1),
                    )
                    if pe_sum:
                        nc.tensor.matmul(
                            ps_sm_acc[:, lo:512],
                            ones16_sb,
                            e[:, lo:512],
                            start=(ki == 0),
                            stop=(ki == nk - 1),
                        )
                        return
                    # denominator partials off the PE:
                    # Pool owns cols [0:256), DVE owns cols [256:512)
                    if lo < 256:
                        if ki == 0:
                            nc.gpsimd.tensor_copy(
                                partial[:, 0:256], e[:, 0:256]
                            )
                        else:
                            nc.gpsimd.tensor_add(
                                partial[:, lo:256],
                                partial[:, lo:256],
                                e[:, lo:256],
                            )
                    hi = max(lo, 256)
                    if ki == 0:
                        nc.vector.tensor_copy(
                            partial[:, 256:512], e[:, 256:512]
                        )
                    else:
                        nc.vector.tensor_add(
                            partial[:, hi:512],
                            partial[:, hi:512],
                            e[:, hi:512],
                        )

                for ki in range(min(LOOKAHEAD, nk)):
                    emit_scores(ki)
                    pull(1)
                for ki in range(nk):
                    emit_pv(ki)
                    pull(2)
                    if ki + LOOKAHEAD < nk:
                        emit_scores(ki + LOOKAHEAD)

                def tail():
                    if pe_sum:
                        ps_sm = ps_sm_acc
                    else:
                        ps_sm = pp.tile([128, 512], F32, tag="ps")
                        nc.tensor.matmul(
                            ps_sm, ones_sb, partial, start=True, stop=True
                        )
                    rc = rcp_pool.tile([128, 512], F32)
                    nc.vector.reciprocal_approx_fast(out=rc, in_=ps_sm)
                    o = out_pool.tile([128, 512], F32)
                    nc.vector.tensor_mul(o, ps_out, rc)
                    nc.scalar.dma_start(
                        out=out_ap[h, :, b, qsl0 : qsl0 + 512], in_=o
                    )

                return tail

            # ---- schedule: proj(b0); then attn(b) interleaved with
            # proj(b+1) pieces; deferred attn tails flush one step later ----
            for _ in proj_stream(0):
                pass

            state = {"gen": None, "count": 0}

            def pull(n):
                for _ in range(n):
                    if state["gen"] is None:
                        return
                    try:
                        next(state["gen"])
                        state["count"] += 1
                    except StopIteration:
                        state["gen"] = None
                        return

            def pull_until(cnt):
                while state["gen"] is not None and state["count"] < cnt:
                    pull(1)

            pending = None  # deferred attn tail, flushed after PE cover
            for b in range(B - 1):
                state["gen"] = proj_stream(b + 1)
                state["count"] = 0
                for h, qs in [(0, 0), (0, 1), (1, 0), (1, 1)]:
                    t = emit_attn_chain(b, h, qs, pull)
                    if pending is not None:
                        pending()
                    pending = t
                if b < B - 2:
                    pull(1 << 30)  # drain the rest of proj(b+1)
                else:
                    # complete proj(B-1) half0 only; half1 pieces are
                    # saved as PE filler for the last batch's qs0 chains
                    # (which depend only on half0)
                    pull_until(128)
            for h, qs in [(0, 0), (1, 0)]:
                t = emit_attn_chain(B - 1, h, qs, pull)
                if pending is not None:
                    pending()
                pending = t
            pull(1 << 30)
            t = emit_attn_chain(B - 1, 0, 1, pull)
            if pending is not None:
                pending()
            pending = t
            # final chain: denominator on PE so the program does not end
            # waiting on the off-engine partial chains
            t2 = emit_attn_chain(B - 1, 1, 1, pull, pe_sum=True)
            if pending is not None:
                pending()
            t2()

    nc.compile()
    return nc


def _prep_shared(hidden_states):
    x2 = np.ascontiguousarray(hidden_states.reshape(T, D).T)  # [D, T]
    # x8[p, bh, kc, s] = x2[kc*128+p, bh*512+s]
    x8 = np.ascontiguousarray(
        x2.reshape(KC, 128, B * NHALF, NSL).transpose(1, 2, 0, 3)
    ).astype(NP_BF16)

    inv = 1.0 / (ROPE_BASE ** (np.arange(0, HD, 2, dtype=np.float64) / HD))
    f = np.outer(inv, np.arange(S, dtype=np.float64))  # [64, S]
    cosT = np.concatenate([np.cos(f), np.cos(f)], axis=0).astype(NP_BF16)
    sinF = np.concatenate([-np.sin(f), np.sin(f)], axis=0).astype(NP_BF16)

    p = np.arange(128)[:, None]
    j = np.arange(128)[None, :]
    tri = (j >= p).astype(NP_BF16)  # keep j >= p in the diagonal window
    ones = np.ones((128, 128), np.float32)
    ones16 = np.ones((128, 128), NP_BF16)
    return x8, cosT, sinF, tri, ones, ones16


def _core_rows(c):
    h0, h1 = 2 * c, 2 * c + 1
    rows = []
    for part in range(3):  # q, k, v blocks
        for h in (h0, h1):
            base = h * 3 * HD + part * HD
            rows.extend(range(base, base + HD))
    return np.asarray(rows)


def _prep_core(w_qkv, b_qkv, c):
    rows = _core_rows(c)
    wsel = w_qkv[rows, :]  # [768, D]; order: q0,q1,k0,k1,v0,v1
    # wqk[p, m, kc, f] = wsel[m*128+f, kc*128+p]
    wqk = np.ascontiguousarray(
        wsel[: 4 * 128, :].reshape(4, 128, KC, 128).transpose(3, 0, 2, 1)
    ).astype(NP_BF16)
    # wv[p, kc, f] = wsel[512+f, kc*128+p]
    wv = np.ascontiguousarray(
        wsel[4 * 128 :, :].reshape(2 * HD, KC, 128).transpose(2, 1, 0)
    ).astype(NP_BF16)
    b_sel = b_qkv[rows]
    bqk = np.ascontiguousarray(
        b_sel[: 4 * 128].reshape(4, 128).T
    ).astype(np.float32)  # [128, 4]
    bv = np.ascontiguousarray(
        np.broadcast_to(b_sel[4 * 128 :], (128, 2 * HD))
    ).astype(np.float32)  # [128, 256]
    return wqk, wv, bqk, bv


def _make_in_maps(hidden_states, w_qkv, b_qkv):
    x8, cosT, sinF, tri, ones, ones16 = _prep_shared(hidden_states)
    in_maps = []
    for c in range(NCORES):
        wqk, wv, bqk, bv = _prep_core(w_qkv, b_qkv, c)
        in_maps.append(
            {
                "x8": x8,
                "wqk": wqk,
                "wv": wv,
                "bqk": bqk,
                "bv": bv,
                "cosT": cosT,
                "sinF": sinF,
                "tri": tri,
                "ones": ones,
                "ones16": ones16,
            }
        )
    return in_maps


def _assemble(results):
    outs = np.stack([results[c]["out"] for c in range(NCORES)])
    # [NCORES, HPC, HD, B, S] -> [B, S, H*HD]
    return np.ascontiguousarray(
        outs.reshape(H, HD, B, S).transpose(2, 3, 0, 1).reshape(B, S, D)
    )


def run(hidden_states, w_qkv, b_qkv, trace=False):
    from concourse.bass_utils import run_bass_kernel_spmd

    if "nc" not in _CACHE:
        _CACHE["nc"] = _build_program()
    nc = _CACHE["nc"]
    in_maps = _make_in_maps(
        np.asarray(hidden_states, dtype=np.float32),
        np.asarray(w_qkv, dtype=np.float32),
        np.asarray(b_qkv, dtype=np.float32),
    )
    res = run_bass_kernel_spmd(
        nc, in_maps, core_ids=list(range(NCORES)), trace=trace
    )
    out = _assemble(res.results)
    return out, res


def kernel(hidden_states, w_qkv, b_qkv):
    trace = os.environ.get("KERNEL_TRACE", "0") == "1"
    out, _res = run(hidden_states, w_qkv, b_qkv, trace=trace)
    return out
